# revision 1
# baseline (speedup 1.0000x reference)
"""Distributed multi-head attention forward on 8 TRN2 NeuronCores.

Problem (hardcoded): x [2, 4096, 512] f32, Wq/Wk/Wv/Wo [512, 512], bo [512].
reference: torch-style MHA with 8 heads of dim 64, softmax scale 1/8.

Sharding: head-parallel. Core h computes head h for BOTH batches:
  - host sends x^T [512, 8192] (bf16) + per-head weight slices (pre-transposed)
  - Q^T/K^T [64, 8192] computed on-chip, duplicated into both partition
    halves so QK^T (contract dim = head_dim 64) runs as two concurrent
    row-tiled matmuls (tile_position (0,0)/(64,0))
  - S^T [j, m] orientation; exp on ScalarE (scale=0.125 fused, no
    max-subtraction: scores ~ N(0,1), max < ~6) over 3-bank PSUM groups,
    double-buffered
  - AV with stationary [V | ones] (M=65): PSUM row 64 = softmax denominator
  - normalize ctx by 1/denom (broadcast via DRAM bounce DMA), stage bf16
  - AllToAll over all 8 cores reshards head-split -> row-split
  - out-proj: full Wo^T per core on its 1024 rows + bias; host concatenates.

Scheduling (guided by TimelineSim engine-occupancy traces):
  - x streams in 512-col chunks after the weight DMAs (weights first so the
    first projection isn't queued behind 8 MiB of x); batch-0 columns first
  - QK-projection for batch 0 runs up front; batch-1 QK/V projections are
    emitted one slice/chunk per quad inside batch-0's ACT-bound blocks
  - flat quad stream: each block's AV matmuls lag its QK by 5 quads, so the
    PE's in-order stream never stalls behind the exp consumer and the next
    block's scores are always in flight at block boundaries
  - A2A is split even/odd m-blocks; the even-half collective and half the
    out-projection overlap the odd blocks' attention
  - small DMAs (dup/recip/staging) ride the otherwise-idle GpSimd queue.
"""

import numpy as np
import ml_dtypes

B, N, C = 2, 4096, 512
H, D = 8, 64
R = B * N            # 8192 global rows
NCORES = 8
MROWS = R // NCORES  # 1024 rows owned per core after A2A
BF16 = ml_dtypes.bfloat16

_CACHE = {}


def _build(reps=1, stages='full', quad=3, s4bufs=2, ctxbufs=2, projbufs=2, ebufs=7, exp_frac=1.0, dve_period=0, lag=5, warmn=2):
    import concourse.bass as bass
    import concourse.tile as tile
    from concourse import bacc, mybir

    import math
    fp32 = mybir.dt.float32
    bf16 = mybir.dt.bfloat16
    i16 = mybir.dt.int16
    SCH_A = float(0.125 * 128.0 / math.log(2.0))   # fold softmax scale
    SCH_B = float(127 * 128 - 0.0579615 * 128)
    AF = mybir.ActivationFunctionType

    nc = bacc.Bacc("TRN2", target_bir_lowering=False, debug=False,
                   num_devices=NCORES)

    xT = nc.dram_tensor("xT", [C, R], bf16, kind="ExternalInput").ap()
    wqk = nc.dram_tensor("wqk", [C, 128], bf16, kind="ExternalInput").ap()
    wv = nc.dram_tensor("wv", [C, D], bf16, kind="ExternalInput").ap()
    wo = nc.dram_tensor("wo", [C, C], bf16, kind="ExternalInput").ap()
    bias = nc.dram_tensor("bias", [128, 4], fp32, kind="ExternalInput").ap()
    out = nc.dram_tensor("out", [C, MROWS], fp32, kind="ExternalOutput").ap()

    KC = C // 128          # 4 contraction chunks of 128 over C
    NJ = N // 128          # 32 key chunks per batch
    MB = 512               # query block width (moving free dim)
    NMB = N // MB          # 8 m-blocks per batch
    QUAD = quad            # j-chunks per exp batch (PSUM banks each)

    with tile.TileContext(nc) as tc:
        with (
            tc.tile_pool(name="xpool", bufs=4) as xpool,
            tc.tile_pool(name="wpool", bufs=1) as wpool,
            tc.tile_pool(name="qk", bufs=1) as qkpool,
            tc.tile_pool(name="vpool", bufs=1) as vpool,
            tc.tile_pool(name="epool", bufs=ebufs) as epool,
            tc.tile_pool(name="stage", bufs=3) as stpool,
            tc.tile_pool(name="misc", bufs=3) as miscpool,
            tc.tile_pool(name="capool", bufs=8) as capool,
            tc.tile_pool(name="ps_s4", bufs=s4bufs, space="PSUM") as ps_s4,
            tc.tile_pool(name="ps_ctx", bufs=ctxbufs, space="PSUM") as ps_ctx,
                        tc.tile_pool(name="dram", bufs=1, space="DRAM") as dram,
        ):
          for _rep in range(reps):
            # ---- load inputs ----
              xt = []
              for k in range(KC):
                  t = xpool.tile([128, R], bf16, tag="xt")
                  xt.append(t)
              wqk_sb = wpool.tile([128, KC, 128], bf16, tag="wqk")
              nc.sync.dma_start(
                  wqk_sb[:], wqk.rearrange("(k p) m -> p k m", p=128))
              wv_sb = wpool.tile([128, KC, D], bf16, tag="wv")
              nc.sync.dma_start(
                  wv_sb[:], wv.rearrange("(k p) m -> p k m", p=128))
              XCH = 512             # x load granularity (columns)
              for c0 in range(0, R, XCH):   # batch-0 chunks land first
                  for k in range(KC):
                      nc.sync.dma_start(
                          xt[k][:, c0:c0 + XCH],
                          xT[k * 128:(k + 1) * 128, c0:c0 + XCH])
              # out-proj weights aren't needed until much later; keep their
              # (slow, strided) loads off the Pool queue that carries the
              # early Q/K duplication DMAs
              wo_sb = wpool.tile([128, KC, C], bf16, tag="wo")
              nc.sync.dma_start(
                  wo_sb[:], wo.rearrange("(k p) m -> p k m", p=128))
              bias_sb = wpool.tile([128, 4], fp32, tag="bias")
              nc.sync.dma_start(bias_sb[:], bias)

              # PE HAM warm-up: the clock gate holds the PE at 1.2 GHz until
              # ~3.4us of sustained activity. Burn dummy matmuls (on the
              # already-resident wqk tile, result discarded) while waiting on
              # DMAs/collectives so real matmuls run at 2.4 GHz.
              def pe_warm(n):
                  wrm = ps_ctx.tile([128, MB], fp32, tag="ctx")
                  for _ in range(n):
                      nc.tensor.matmul(
                          wrm[:], wqk_sb[:, 0, :],
                          wqk_sb.rearrange("p k m -> p (k m)"),
                          start=True, stop=True, skip_group_check=True)

              if warmn:
                  pe_warm(warmn)   # sized to fit inside the x-DMA wait

              # ---- QK projection: psum = [Q^T (parts 0:64); K^T (parts 64:128)]
              qt2 = qkpool.tile([128, R], bf16, tag="qt2")   # Q^T in both halves
              kt2 = qkpool.tile([128, R], bf16, tag="kt2")   # K^T in both halves
              def qk_proj(ms):
                  # one 512-wide slice: project, evict both halves, then
                  # duplicate this slice into the opposite partition halves
                  ps = ps_ctx.tile([128, MB], fp32, tag="ctx")
                  for k in range(KC):
                      nc.tensor.matmul(
                          ps[:], wqk_sb[:, k, :],
                          xt[k][:, ms * MB:(ms + 1) * MB],
                          start=(k == 0), stop=(k == KC - 1))
                  sl = slice(ms * MB, (ms + 1) * MB)
                  nc.vector.tensor_copy(qt2[0:64, sl], ps[0:64, :])
                  nc.vector.tensor_copy(kt2[64:128, sl], ps[64:128, :])
                  nc.gpsimd.dma_start(qt2[64:128, sl], qt2[0:64, sl])
                  nc.gpsimd.dma_start(kt2[0:64, sl], kt2[64:128, sl])

              for ms in range(4):       # slices 4..7 + batch 1 are JIT'd
                  qk_proj(ms)

              # ---- V storage: V natural [j, 64] + ones column (col 64).
              # Projection matmuls are emitted just-in-time inside the first
              # m-block of each batch (fills PE slack under the ACT-bound
              # attention steady state).
              vst = vpool.tile([128, 2 * NJ, D + 1], bf16, tag="vst")
              nc.vector.memset(vst[:, :, D:D + 1], 1.0)

              def v_proj(jc):
                  psv = ps_ctx.tile([128, MB], fp32, tag="ctx")
                  ps = psv[:, 0:D]
                  for k in range(KC):
                      nc.tensor.matmul(
                          ps[:], xt[k][:, jc * 128:(jc + 1) * 128],
                          wv_sb[:, k, :],
                          start=(k == 0), stop=(k == KC - 1))
                  nc.vector.tensor_copy(vst[:, jc, 0:D], ps[:])

              if stages == 'proj':
                  for jc in range(2 * NJ):
                      v_proj(jc)
                  continue

              # ---- attention + A2A staging (split into two half-collectives:
              # even m-blocks -> half A, odd -> half B, so A2A(A) and the
              # first half of out-proj overlap the odd m-blocks' attention) --
              a2a = [dram.tile([R // 16, MB], bf16, name=f"a2a_in{i}")
                     for i in range(2)]
              a2a_o = [dram.tile([R // 16, MB], bf16, name=f"a2a_out{i}")
                       for i in range(2)]
              rec_d = dram.tile([16, MB], fp32)            # recip bounce rows

              def mk_block(b, mb, fill=None):
                  # Returns (qk_thunks, av_thunks, tail): the driver emits
                  # qk(t+1) before av(t) so the PE stream always has the next
                  # quad's scores in flight when a block ends (keeps ACT fed
                  # across block boundaries).
                  msl = slice(b * N + mb * MB, b * N + (mb + 1) * MB)
                  state = {"ctx": None, "first": True}

                  def get_ctx():
                      if state["ctx"] is None:
                          ctxf = ps_ctx.tile([128, MB], fp32, tag="ctx")
                          state["ctx"] = ctxf[0:D + 1, :]
                      return state["ctx"]

                  def mk_qk(q0, nq):
                      # every dve_period-th quad computes exp on VectorE via
                      # the Schraudolph bit trick (one mult+add rounded into
                      # int16 == bf16 bits of exp), offloading the saturated
                      # ScalarE; ~1.8% elementwise rel err on those chunks
                      on_dve = dve_period and ((q0 // QUAD) % dve_period
                                               == dve_period - 1)

                      def qk():
                          s4 = ps_s4.tile([128, QUAD * MB], fp32, tag="s4")
                          for qi in range(nq):
                              jc = q0 + qi
                              half = 64 * (jc % 2)   # global alternation:
                              # consecutive chunks always use opposite PE
                              # row-halves, so every adjacent pair can run
                              # concurrently (incl. across quad boundaries)
                              jsl = slice(b * N + jc * 128,
                                          b * N + (jc + 1) * 128)
                              nc.tensor.matmul(
                                  s4[:, qi * MB:(qi + 1) * MB],
                                  kt2[half:half + 64, jsl],
                                  qt2[half:half + 64, msl],
                                  start=True, stop=True,
                                  tile_position=(half, 0))
                          if on_dve:
                              e4i = epool.tile([128, QUAD * MB], i16,
                                               tag="e4i")
                              nc.vector.tensor_scalar(
                                  out=e4i[:, 0:nq * MB],
                                  in0=s4[:, 0:nq * MB],
                                  scalar1=SCH_A, scalar2=SCH_B,
                                  op0=mybir.AluOpType.mult,
                                  op1=mybir.AluOpType.add)
                              e4 = e4i[:].bitcast(bf16)
                          else:
                              e4 = epool.tile([128, QUAD * MB], bf16,
                                              tag="e4")
                              nexp = max(1, int(nq * MB * exp_frac)) \
                                  // 128 * 128
                              nc.scalar.activation(e4[:, 0:nexp],
                                                   s4[:, 0:nexp], AF.Exp,
                                                   scale=0.125)
                          if b == 0 and mb == 0:  # JIT V proj for batch 0
                              for qi in range(nq):
                                  v_proj(q0 + qi)
                          if fill is not None:
                              fill()
                          return e4
                      return qk

                  def mk_av(q0, nq):
                      def av(e4):
                          ctx = get_ctx()
                          for qi in range(nq):
                              jc = q0 + qi
                              nc.tensor.matmul(
                                  ctx[:], vst[:, b * NJ + jc, :],
                                  e4[:, qi * MB:(qi + 1) * MB],
                                  start=state["first"], stop=(jc == NJ - 1),
                                  skip_group_check=True)
                              state["first"] = False
                      return av

                  qks, avs = [], []
                  for q0 in range(0, NJ, QUAD):
                      nq = min(QUAD, NJ - q0)
                      qks.append(mk_qk(q0, nq))
                      avs.append(mk_av(q0, nq))

                  def tail():
                      ctx = state["ctx"]
                      # normalize: recip of denom row, broadcast via DRAM
                      rid = b * NMB + mb
                      rc = miscpool.tile([1, MB], fp32, tag="rc")
                      nc.vector.reciprocal(rc[:], ctx[D:D + 1, :])
                      # block tails run long after the x loads: the sync
                      # queue (fast HWDGE issue, ~0.65us) is idle by then,
                      # vs ~2.5-4us SWDGE descriptor-gen on the Pool queue
                      nc.sync.dma_start(rec_d[rid:rid + 1, :], rc[:])
                      rb = miscpool.tile([64, MB], fp32, tag="rb")
                      bcast = bass.AP(
                          tensor=rec_d.tensor,
                          offset=rec_d[rid:rid + 1, :].offset,
                          ap=[[0, 64]] + rec_d[rid:rid + 1, :].ap[1:])
                      nc.sync.dma_start(rb[:], bcast)
                      st = stpool.tile([64, MB], bf16, tag="st")
                      nc.vector.tensor_mul(st[:], ctx[0:D, :], rb[:])
                      # shard s covers global rows [s*1024, (s+1)*1024)
                      s = (b * N + mb * MB) // MROWS
                      nc.sync.dma_start(
                          a2a[mb % 2][s * 64:(s + 1) * 64, :], st[:])
                  return qks, avs, tail

              def a2a_half(p):
                  nc.gpsimd.collective_compute(
                      "AllToAll", mybir.AluOpType.bypass,
                      replica_groups=[list(range(NCORES))],
                      ins=[a2a[p].opt()], outs=[a2a_o[p].opt()])

              def outproj_half(p):
                  ca = []
                  for k in range(KC):
                      t = capool.tile([128, MB], bf16, tag="ca")
                      nc.sync.dma_start(
                          t[:], a2a_o[p][k * 128:(k + 1) * 128, :])
                      ca.append(t)
                  for cc in range(KC):
                      ps = ps_ctx.tile([128, MB], fp32, tag="ctx")
                      for k in range(KC):
                          nc.tensor.matmul(
                              ps[:], wo_sb[:, k, cc * 128:(cc + 1) * 128],
                              ca[k][:], start=(k == 0), stop=(k == KC - 1))
                      ot = stpool.tile([128, MB], fp32, tag="ot")
                      nc.vector.tensor_scalar_add(ot[:], ps[:],
                                                  bias_sb[:, cc:cc + 1])
                      nc.sync.dma_start(
                          out[cc * 128:(cc + 1) * 128,
                              p * MB:(p + 1) * MB], ot[:])

              # batch-1 QK+V projections dribble into b0's ACT-bound middle
              # blocks (qkproj slices first -- b1 attention needs them at
              # idx 4 -- then V chunks, one item per quad)
              fill_items = [lambda s=s: qk_proj(NMB + s) for s in range(NMB)]
              fill_items += [lambda jc=jc: v_proj(NJ + jc) for jc in range(NJ)]
              f_ctr = [0]

              def v1_fill():   # one item per quad: 40 items over 5 blocks
                  if f_ctr[0] < len(fill_items):
                      fill_items[f_ctr[0]]()
                      f_ctr[0] += 1

              b0_items = [lambda s=s: qk_proj(s) for s in range(4, NMB)]
              b0_ctr = [0]

              def b0_fill():   # rest of batch-0 QK proj inside block 0
                  if b0_ctr[0] < len(b0_items):
                      b0_items[b0_ctr[0]]()
                      b0_ctr[0] += 1

              order = [(b, mb) for par in (0, 1) for b in range(B)
                       for mb in range(par, NMB, 2)]
              # flat quad stream with AV lagging QK by one quad
              stream = []   # (qk, av, after_fn)
              for idx, (b, mb) in enumerate(order):
                  fillfn = None
                  if idx == 0:
                      fillfn = b0_fill
                  elif idx in (1, 2, 3, 4, 5):
                      fillfn = v1_fill
                  qks, avs, tail = mk_block(b, mb, fill=fillfn)
                  after = [None] * len(qks)
                  post = [tail]
                  if stages != 'attn':
                      if idx == 7:
                          post.append(lambda: a2a_half(0))
                      elif idx == 11:
                          post.append(lambda: outproj_half(0))
                      elif idx == 15:
                          post.append(lambda: (a2a_half(1), pe_warm(8),
                                               outproj_half(1)))
                  after[-1] = post
                  stream.extend(zip(qks, avs, after))

              from collections import deque
              pending = deque()

              def flush_one():
                  pav, pe4, pafter = pending.popleft()
                  pav(pe4)
                  if pafter:
                      for fn in pafter:
                          fn()

              for qk, av, after in stream:
                  e4 = qk()
                  if len(pending) >= lag:
                      flush_one()
                  pending.append((av, e4, after))
              while pending:
                  flush_one()

    nc.compile()
    return nc


def _prep_inputs(x, Wq, Wk, Wv, Wo, bo):
    x = np.asarray(x, np.float32)
    Wq = np.asarray(Wq, np.float32)
    Wk = np.asarray(Wk, np.float32)
    Wv = np.asarray(Wv, np.float32)
    Wo = np.asarray(Wo, np.float32)
    bo = np.asarray(bo, np.float32)

    xT = np.ascontiguousarray(x.reshape(R, C).T).astype(BF16)
    woT = np.ascontiguousarray(Wo.T).astype(BF16)
    bias = np.ascontiguousarray(bo.reshape(4, 128).T).astype(np.float32)

    in_maps = []
    for h in range(NCORES):
        sl = slice(h * D, (h + 1) * D)
        wqk = np.concatenate(
            [Wq[sl].T, Wk[sl].T], axis=1).astype(BF16)
        wv = np.ascontiguousarray(Wv[sl].T).astype(BF16)
        in_maps.append({
            "xT": xT,
            "wqk": np.ascontiguousarray(wqk),
            "wv": wv,
            "wo": woT,
            "bias": bias,
        })
    return in_maps


def kernel(x, Wq, Wk, Wv, Wo, bo, _want_results=False, _trace=False):
    from concourse import bass_utils

    if "nc" not in _CACHE:
        _CACHE["nc"] = _build(1)
    nc = _CACHE["nc"]

    in_maps = _prep_inputs(x, Wq, Wk, Wv, Wo, bo)
    res = bass_utils.run_bass_kernel_spmd(
        nc, in_maps, core_ids=list(range(NCORES)), trace=_trace)

    outT = np.concatenate(
        [np.asarray(res.results[j]["out"]) for j in range(NCORES)], axis=1)
    full = np.ascontiguousarray(outT.T).reshape(B, N, C).astype(np.float32)
    if _want_results:
        return full, res
    return full


def bench(x, Wq, Wk, Wv, Wo, bo, iters=8, reps=3, body_reps=1, nc=None):
    """Measure per-NEFF-execution time by chaining `iters` executions in one
    jit (output of exec i feeds the donated out-buffer operand of exec i+1),
    so per-exec time = (t_chain(iters) - t_chain(1)) / (iters - 1)."""
    import time
    import jax
    from jax.experimental.shard_map import shard_map
    from jax.sharding import Mesh, PartitionSpec
    from concourse import bass2jax, mybir

    if nc is None:
        key = ("nc", body_reps)
        if key not in _CACHE:
            _CACHE[key] = _build(body_reps)
        nc = _CACHE[key]
    bass2jax.install_neuronx_cc_hook()

    in_maps = _prep_inputs(x, Wq, Wk, Wv, Wo, bo)

    pname = nc.partition_id_tensor.name if nc.partition_id_tensor else None
    in_names, out_names, out_avals = [], [], []
    for alloc in nc.m.functions[0].allocations:
        if not isinstance(alloc, mybir.MemoryLocationSet):
            continue
        name = alloc.memorylocations[0].name
        if alloc.kind == "ExternalInput":
            if name != pname:
                in_names.append(name)
        elif alloc.kind == "ExternalOutput":
            out_names.append(name)
            out_avals.append(jax.core.ShapedArray(
                tuple(alloc.tensor_shape), mybir.dt.np(alloc.dtype)))
    n_params = len(in_names)
    all_names = in_names + out_names + ([pname] if pname else [])

    def _body(*args):
        ins = list(args[:n_params])
        outs = list(args[n_params:])
        extra = [bass2jax.partition_id_tensor()] if pname else []
        outs = list(bass2jax._bass_exec_p.bind(
            *ins, *outs, *extra,
            out_avals=tuple(out_avals),
            in_names=tuple(all_names),
            out_names=tuple(out_names),
            lowering_input_output_aliases=(),
            sim_require_finite=True,
            sim_require_nnan=True,
            nc=nc))
        return tuple(outs)

    devices = jax.devices()[:NCORES]
    mesh = Mesh(np.asarray(devices), ("core",))
    specs = (PartitionSpec("core"),) * (n_params + len(out_names))
    ospecs = (PartitionSpec("core"),) * len(out_names)
    fn = jax.jit(shard_map(_body, mesh=mesh, in_specs=specs,
                           out_specs=ospecs, check_rep=False))

    concat_in = [np.concatenate([np.asarray(in_maps[c][n])[None]
                                 for c in range(NCORES)], axis=0)
                 .reshape(NCORES * in_maps[0][n].shape[0],
                          *in_maps[0][n].shape[1:])
                 for n in in_names]
    concat_zero = [np.zeros((NCORES * a.shape[0], *a.shape[1:]), a.dtype)
                   for a in out_avals]
    dev_in = [jax.device_put(a) for a in concat_in]
    dev_zero = [jax.device_put(a) for a in concat_zero]

    fn(*dev_in, *dev_zero)[0].block_until_ready()  # compile+warm

    def chain(k):
        outs = tuple(dev_zero)
        t0 = time.perf_counter()
        for _ in range(k):
            outs = fn(*dev_in, *outs)
        outs[0].block_until_ready()
        return time.perf_counter() - t0

    ts = [chain(iters) for _ in range(reps)]
    t = min(ts)
    print(f"body_reps={body_reps} chain k={iters}: min {t*1e6:.0f} us")
    return t



# revision 48
# speedup vs baseline: 1.2987x; 1.2987x over previous
"""Distributed multi-head attention forward on 8 TRN2 NeuronCores.

Problem (hardcoded): x [2, 4096, 512] f32, Wq/Wk/Wv/Wo [512, 512], bo [512].
reference: torch-style MHA with 8 heads of dim 64, softmax scale 1/8.

Sharding: head-parallel. Core h computes head h for BOTH batches:
  - host sends x^T [512, 8192] (bf16) + per-head weight slices (pre-transposed)
  - Q^T/K^T [64, 8192] computed on-chip, duplicated into both partition
    halves so QK^T (contract dim = head_dim 64) runs as two concurrent
    row-tiled matmuls (tile_position (0,0)/(64,0))
  - S^T [j, m] orientation; exp is split across ScalarE (native Exp, fused
    scale) and VectorE (Schraudolph bit trick: one mult+add rounded into
    int16 == bf16 bits of exp, ~1.8% rms on those chunks), greedily
    balanced against each engine's other duties
  - AV runs TRANSPOSED: exp output e4 [j,m] is the 128x128 stationary
    operand, [V | ones] [128, 65] is the moving operand, accumulating
    ctx^T [m, d+1] in PSUM (65-cycle matmuls instead of 512)
  - normalize: denominator is per-PARTITION in ctx^T, so a reciprocal +
    per-partition tensor_scalar multiply normalizes directly (no DRAM
    broadcast bounce); a PE transpose (vs host-fed identity) restores
    [d, m] for A2A staging
  - AllToAll over all 8 cores reshards head-split -> row-split in THREE
    pieces (even m-blocks [512 wide], then odd blocks in two 256-wide
    column phases), so each collective starts as soon as its phase's
    blocks finish and the post-attention tail is one small piece
    (collective cost = 15us constant + size/40GBps)
  - out-proj per piece: full Wo^T on its rows + bias; host concatenates.

Scheduling (guided by TimelineSim engine-occupancy traces + HW bisects):
  - x streams in 512-col chunks after the weight DMAs; batch-0 first
  - QK-projection for batch 0 runs up front; batch-1 QK/V projections are
    emitted one item per quad inside batch-0's early blocks, with V
    projected 4 chunks per PSUM tile so the fill chain (matmuls -> evict
    -> next alloc through the one free PSUM buf) never head-blocks the
    in-order PE queue
  - flat quad stream: AV lags QK by `lag` quads; block tails are split
    (normalize at +0, transpose/stage at +2 quads) so the PE never waits
    on the DVE normalize
  - narrow blocks take proportionally more j-chunks per exp quad (fd
    stays 1024) and alternate QK row-halves per chunk-GROUP: per-chunk
    ping-pong of tile_position at <512 moving cols hard-faults the HW
  - each block's ctx^T and its transposed staging view share one PSUM
    bank (bf16 bitcast of the upper half), and only the block's first AV
    matmul sets start=True -- start zeroes the whole 2KB zero-region
  - mid-stream out-projections are emitted one chunk per block; the
    out-proj of the last two pieces runs in the drain
  - small DMAs (Q/K duplication) ride the otherwise-idle GpSimd queue.
"""

import numpy as np
import ml_dtypes

B, N, C = 2, 4096, 512
H, D = 8, 64
R = B * N            # 8192 global rows
NCORES = 8
MROWS = R // NCORES  # 1024 rows owned per core after A2A
BF16 = ml_dtypes.bfloat16

_CACHE = {}

# A2A piece column phases (widths must be multiples of 128): piece p covers
# out columns [sum(PIECEW[:p]), +PIECEW[p]) of each core's 1024 rows; uniform
# 256-wide phases let every collective start early and keep exp instructions
# full-width
PIECEW = (512, 256, 256)


def _build(reps=1, stages='full', quad=2, s4bufs=3, ctxbufs=2, ebufs=9,
           lag=5, warmn=2, dve_bias=0.0, nblk=None, debug_blk=None):
    import concourse.bass as bass
    import concourse.tile as tile
    from concourse import bacc, mybir

    import math
    fp32 = mybir.dt.float32
    bf16 = mybir.dt.bfloat16
    i16 = mybir.dt.int16
    SCH_A = float(0.125 * 128.0 / math.log(2.0))   # fold softmax scale
    SCH_B = float(127 * 128 - 0.0579615 * 128)
    AF = mybir.ActivationFunctionType

    nc = bacc.Bacc("TRN2", target_bir_lowering=False, debug=False,
                   num_devices=NCORES)

    xT = nc.dram_tensor("xT", [C, R], bf16, kind="ExternalInput").ap()
    wqk = nc.dram_tensor("wqk", [C, 128], bf16, kind="ExternalInput").ap()
    wv = nc.dram_tensor("wv", [C, D], bf16, kind="ExternalInput").ap()
    wo = nc.dram_tensor("wo", [C, C], bf16, kind="ExternalInput").ap()
    bias = nc.dram_tensor("bias", [128, 4], fp32, kind="ExternalInput").ap()
    ident = nc.dram_tensor("ident", [128, 128], bf16,
                           kind="ExternalInput").ap()
    out = nc.dram_tensor("out", [C, MROWS], fp32, kind="ExternalOutput").ap()

    KC = C // 128          # 4 contraction chunks of 128 over C
    NJ = N // 128          # 32 key chunks per batch
    MB = 512               # base m-block width
    NMB = N // MB          # 8 m-blocks per batch
    QUAD = quad            # j-chunks per exp batch

    # running busy-time estimates for the exp-engine balancer (ns)
    eng_busy = {"A": 1283.0, "D": 0.0}   # ACT starts with the table load

    def act_cost(fd):
        return (fd + 222) * 0.8333

    def dve_cost(fd):
        return (fd + 120) * 1.0417

    with tile.TileContext(nc) as tc:
        with (
            tc.tile_pool(name="xpool", bufs=4) as xpool,
            tc.tile_pool(name="wpool", bufs=1) as wpool,
            tc.tile_pool(name="qk", bufs=1) as qkpool,
            tc.tile_pool(name="vpool", bufs=1) as vpool,
            tc.tile_pool(name="epool", bufs=ebufs) as epool,
            tc.tile_pool(name="stage", bufs=3) as stpool,
            tc.tile_pool(name="misc", bufs=3) as miscpool,
            tc.tile_pool(name="capool", bufs=8) as capool,
            tc.tile_pool(name="ps_s4", bufs=s4bufs, space="PSUM") as ps_s4,
            tc.tile_pool(name="ps_ctx", bufs=ctxbufs, space="PSUM") as ps_ctx,
            tc.tile_pool(name="dram", bufs=1, space="DRAM") as dram,
        ):
          for _rep in range(reps):
            # ---- load inputs ----
              xt = []
              for k in range(KC):
                  t = xpool.tile([128, R], bf16, tag="xt")
                  xt.append(t)
              wqk_sb = wpool.tile([128, KC, 128], bf16, tag="wqk")
              nc.sync.dma_start(
                  wqk_sb[:], wqk.rearrange("(k p) m -> p k m", p=128))
              wv_sb = wpool.tile([128, KC, D], bf16, tag="wv")
              nc.sync.dma_start(
                  wv_sb[:], wv.rearrange("(k p) m -> p k m", p=128))
              id_sb = wpool.tile([128, 128], bf16, tag="ident")
              nc.sync.dma_start(id_sb[:], ident)
              XCH = 512             # x load granularity (columns)
              for c0 in range(0, R, XCH):   # batch-0 chunks land first
                  for k in range(KC):
                      nc.sync.dma_start(
                          xt[k][:, c0:c0 + XCH],
                          xT[k * 128:(k + 1) * 128, c0:c0 + XCH])
              # out-proj weights aren't needed until much later; keep their
              # (slow, strided) loads off the Pool queue that carries the
              # early Q/K duplication DMAs
              wo_sb = wpool.tile([128, KC, C], bf16, tag="wo")
              nc.sync.dma_start(
                  wo_sb[:], wo.rearrange("(k p) m -> p k m", p=128))
              bias_sb = wpool.tile([128, 4], fp32, tag="bias")
              nc.sync.dma_start(bias_sb[:], bias)

              # PE HAM warm-up: the clock gate holds the PE at 1.2 GHz until
              # ~3.4us of sustained activity. Burn dummy matmuls (on a
              # memset tile, result discarded) while waiting on DMAs /
              # collectives so real matmuls run at 2.4 GHz; the memset
              # source lets the PE start at t~0 instead of after the first
              # weight DMA lands.
              wrm_src = miscpool.tile([128, MB], bf16, tag="wsrc")
              nc.vector.memset(wrm_src[:], 0.5)

              def pe_warm(n):
                  wrm = ps_ctx.tile([128, MB], fp32, tag="ctx")
                  for _ in range(n):
                      nc.tensor.matmul(
                          wrm[:], wrm_src[:, 0:128], wrm_src[:],
                          start=True, stop=True, skip_group_check=True)

              if warmn:
                  pe_warm(warmn)   # sized to fit inside the x-DMA wait

              # ---- QK projection: psum = [Q^T (parts 0:64); K^T (parts 64:128)]
              qt2 = qkpool.tile([128, R], bf16, tag="qt2")   # Q^T in both halves
              kt2 = qkpool.tile([128, R], bf16, tag="kt2")   # K^T in both halves
              def qk_proj(ms):
                  # one 512-wide slice: project, evict both halves (ScalarE;
                  # VectorE is loaded with exp work), then duplicate this
                  # slice into the opposite partition halves
                  ps = ps_ctx.tile([128, MB], fp32, tag="ctx")
                  for k in range(KC):
                      nc.tensor.matmul(
                          ps[:], wqk_sb[:, k, :],
                          xt[k][:, ms * MB:(ms + 1) * MB],
                          start=(k == 0), stop=(k == KC - 1))
                  sl = slice(ms * MB, (ms + 1) * MB)
                  nc.scalar.copy(qt2[0:64, sl], ps[0:64, :])
                  nc.scalar.copy(kt2[64:128, sl], ps[64:128, :])
                  eng_busy["A"] += 2 * act_cost(MB)
                  nc.gpsimd.dma_start(qt2[64:128, sl], qt2[0:64, sl])
                  nc.gpsimd.dma_start(kt2[0:64, sl], kt2[64:128, sl])

              for ms in range(4):       # slices 4..7 + batch 1 are JIT'd
                  qk_proj(ms)

              # ---- V storage: V natural [j, 64] + ones column (col 64).
              vst = vpool.tile([128, 2 * NJ, D + 1], bf16, tag="vst")
              nc.vector.memset(vst[:, :, D:D + 1], 1.0)

              def v_proj_group(jg):
                  # project 4 j-chunks into one PSUM tile + one strided
                  # eviction: a single-tile chain (matmuls -> evict -> next
                  # alloc) is 4x shorter than per-chunk tiles and stops the
                  # fills from head-blocking the in-order PE queue
                  psv = ps_ctx.tile([128, MB], fp32, tag="ctx")
                  for jj in range(4):
                      jc = jg * 4 + jj
                      ps = psv[:, jj * D:(jj + 1) * D]
                      for k in range(KC):
                          nc.tensor.matmul(
                              ps[:], xt[k][:, jc * 128:(jc + 1) * 128],
                              wv_sb[:, k, :],
                              start=(k == 0), stop=(k == KC - 1))
                  nc.scalar.copy(vst[:, jg * 4:(jg + 1) * 4, 0:D],
                                 psv[:, 0:4 * D])
                  eng_busy["A"] += act_cost(4 * D)

              if stages == 'proj':
                  for jg in range(2 * NJ // 4):
                      v_proj_group(jg)
                  continue

              # ---- A2A pieces ----
              a2a = [dram.tile([R // 16, w], bf16, name=f"a2a_in{i}")
                     for i, w in enumerate(PIECEW)]
              a2a_o = [dram.tile([R // 16, w], bf16, name=f"a2a_out{i}")
                       for i, w in enumerate(PIECEW)]
              if debug_blk is not None:
                  dbg_e4 = dram.tile([128, 1024], bf16, name="dbg_e4")
                  dbg_ctx = dram.tile([128, 260], fp32, name="dbg_ctx")
                  dbg_st = dram.tile([128, 256], bf16, name="dbg_st")

              def mk_block(b, mb, c0, w, piece, fill=None):
                  # Returns (qk_thunks, av_thunks, tail_a, tail_b). Block
                  # covers query columns [mb*MB+c0, mb*MB+c0+w) of batch b.
                  # Narrow blocks take more j-chunks per exp so every exp
                  # instruction stays [128, quad*512] (per-inst overhead
                  # would otherwise dominate the 128-wide phases).
                  base = b * N + mb * MB + c0
                  nsub = w // 128
                  QN = (quad * MB) // w      # j-chunks per exp quad
                  state = {"ctx": None, "first": True}

                  def get_ctx():
                      if state["ctx"] is None:
                          # one PSUM bank per block: fp32 cols 0:260 hold
                          # ctx^T [128 m, 4x65]; the bank's upper half
                          # (bf16 view) receives the PE-transposed [d, m]
                          ctxf = ps_ctx.tile([128, MB], fp32, tag="ctx")
                          state["ctx"] = ctxf
                      return state["ctx"]

                  def mk_qk(q0, nq):
                      def qk():
                          s4 = ps_s4.tile([128, QUAD * MB], fp32, tag="s4")
                          grp = max(1, MB // w)
                          for qi in range(nq):
                              jc = q0 + qi
                              half = 64 * ((jc // grp) % 2)
                              # consecutive chunk-groups use opposite PE
                              # row-halves so adjacent groups can run
                              # concurrently (incl. across quad boundaries);
                              # grouping keeps the half-switch period at
                              # >=512 moving cols -- per-chunk ping-pong at
                              # 256-wide hard-faults the device
                              jsl = slice(b * N + jc * 128,
                                          b * N + (jc + 1) * 128)
                              nc.tensor.matmul(
                                  s4[:, qi * w:(qi + 1) * w],
                                  kt2[half:half + 64, jsl],
                                  qt2[half:half + 64, base:base + w],
                                  start=True, stop=True,
                                  tile_position=(half, 0))
                          # exp: greedily balance ScalarE vs VectorE busy
                          fd = nq * w
                          ca, cd = act_cost(fd), dve_cost(fd)
                          on_dve = (eng_busy["D"] + cd + dve_bias
                                    < eng_busy["A"] + ca)
                          if on_dve:
                              eng_busy["D"] += cd
                              e4i = epool.tile([128, QUAD * MB], i16,
                                               tag="e4")
                              nc.vector.tensor_scalar(
                                  out=e4i[:, 0:fd],
                                  in0=s4[:, 0:fd],
                                  scalar1=SCH_A, scalar2=SCH_B,
                                  op0=mybir.AluOpType.mult,
                                  op1=mybir.AluOpType.add)
                              e4 = e4i[:].bitcast(bf16)
                          else:
                              eng_busy["A"] += ca
                              e4t = epool.tile([128, QUAD * MB], bf16,
                                               tag="e4")
                              e4 = e4t[:]
                              nc.scalar.activation(e4[:, 0:fd],
                                                   s4[:, 0:fd], AF.Exp,
                                                   scale=0.125)
                          if b == 0 and mb == 0 and c0 == 0 and q0 % 4 == 0:
                              v_proj_group(q0 // 4)  # JIT V proj, batch 0
                          if fill is not None:
                              fill()
                          if debug_blk == (b, mb, c0) and q0 == 0:
                              nc.sync.dma_start(dbg_e4[:, 0:fd],
                                                e4[:, 0:fd])
                          return e4
                      return qk

                  def mk_av(q0, nq):
                      def av(e4):
                          if stages == 'qkexp':
                              return
                          ctx = get_ctx()
                          for qi in range(nq):
                              jc = q0 + qi
                              vsl = vst[:, b * NJ + jc, :]
                              for sub in range(nsub):
                                  # start=True zeroes the ENTIRE 2KB psum
                                  # zero-region, so only the block's very
                                  # first AV matmul may set it; the other
                                  # subs' first writes land on the pending-
                                  # zero region and overwrite cleanly
                                  nc.tensor.matmul(
                                      ctx[:, sub * 65:sub * 65 + 65],
                                      e4[:, qi * w + sub * 128:
                                         qi * w + (sub + 1) * 128],
                                      vsl,
                                      start=state["first"],
                                      stop=(jc == NJ - 1),
                                      skip_group_check=True)
                                  state["first"] = False
                      return av

                  qks, avs = [], []
                  for q0 in range(0, NJ, QN):
                      nq = min(QN, NJ - q0)
                      qks.append(mk_qk(q0, nq))
                      avs.append(mk_av(q0, nq))

                  def tail_a():
                      if stages in ('qkexp', 'noav', 'notail'):
                          return
                      if debug_blk == (b, mb, c0):
                          dbc = stpool.tile([128, 260], fp32, tag="dbg")
                          nc.vector.tensor_copy(
                              dbc[:], state["ctx"][:, 0:260])
                          nc.sync.dma_start(dbg_ctx[:], dbc[:])
                      ctx = state["ctx"]
                      # normalize in ctx^T orientation: denominator is
                      # per-partition -> reciprocal + per-partition multiply
                      st = stpool.tile([128, nsub * D], bf16, tag="st")
                      state["st"] = st
                      for sub in range(nsub):
                          rc = miscpool.tile([128, 1], fp32, tag="rc")
                          nc.vector.reciprocal(
                              rc[:], ctx[:, sub * 65 + D:sub * 65 + D + 1])
                          nc.vector.tensor_scalar_mul(
                              st[:, sub * D:(sub + 1) * D],
                              ctx[:, sub * 65:sub * 65 + D], rc[:, 0:1])
                          eng_busy["D"] += 126 + dve_cost(D)

                  def tail_b():
                      if stages in ('qkexp', 'noav', 'notail', 'taila'):
                          return
                      # runs two quads later so the PE queue never waits on
                      # the DVE normalize: transpose back to [d, m] via PE
                      # (vs identity) into the ctx bank's upper half, evict,
                      # stage to the A2A buffer
                      ctx, st = state["ctx"], state["st"]
                      pt = ctx[0:64, 256:512].bitcast(bf16)  # [64, 512]
                      for sub in range(nsub):
                          nc.tensor.transpose(
                              pt[:, sub * 128:(sub + 1) * 128],
                              st[:, sub * D:(sub + 1) * D], id_sb[:])
                      sg = stpool.tile([64, w], bf16, tag="sg")
                      nc.scalar.copy(sg[:], pt[:, 0:w])
                      eng_busy["A"] += act_cost(w)
                      if debug_blk == (b, mb, c0):
                          nc.sync.dma_start(dbg_st[:, 0:nsub * D], st[:])
                      # shard s owns global rows [s*1024, (s+1)*1024)
                      s = (b * NMB + mb) // 2
                      nc.sync.dma_start(
                          a2a[piece][s * 64:(s + 1) * 64, :], sg[:])
                  return qks, avs, tail_a, tail_b

              def a2a_piece(p):
                  nc.gpsimd.collective_compute(
                      "AllToAll", mybir.AluOpType.bypass,
                      replica_groups=[list(range(NCORES))],
                      ins=[a2a[p].opt()], outs=[a2a_o[p].opt()])

              # out-column ranges per piece (even block 0:512, odd 512:1024)
              PIECE_OUT = []
              off = 0
              for i, w in enumerate(PIECEW):
                  PIECE_OUT.append((off, w))
                  off += w

              def outproj_items(p, drain=False):
                  # split into 5 thunks (ca loads + one per out-chan chunk)
                  # so the single-free-PSUM-buf chain never head-blocks the
                  # in-order PE queue for more than ~1 quad. In the drain
                  # (last pieces) the ACT/DVE queues are idle: spread the
                  # DMA issue across them so 625ns HWDGE issues overlap.
                  o0, w = PIECE_OUT[p]
                  ca = []
                  qs = [nc.sync] * 4

                  def loads():
                      for k in range(KC):
                          t = capool.tile([128, MB], bf16, tag="ca")
                          qs[k].dma_start(
                              t[:, 0:w], a2a_o[p][k * 128:(k + 1) * 128, :])
                          ca.append(t)

                  def mk_cc(cc):
                      def cc_fn():
                          ps = ps_ctx.tile([128, MB], fp32, tag="ctx")
                          for k in range(KC):
                              nc.tensor.matmul(
                                  ps[:, 0:w],
                                  wo_sb[:, k, cc * 128:(cc + 1) * 128],
                                  ca[k][:, 0:w], start=(k == 0),
                                  stop=(k == KC - 1))
                          ot = stpool.tile([128, MB], fp32, tag="ot")
                          nc.vector.tensor_scalar_add(
                              ot[:, 0:w], ps[:, 0:w], bias_sb[:, cc:cc + 1])
                          eng_busy["D"] += dve_cost(w)
                          qs[cc].dma_start(
                              out[cc * 128:(cc + 1) * 128, o0:o0 + w],
                              ot[:, 0:w])
                      return cc_fn

                  def first():
                      loads()
                      mk_cc(0)()
                  return [first] + [mk_cc(cc) for cc in range(1, KC)]

              def outproj_piece(p):
                  for fn in outproj_items(p):
                      fn()

              # batch-1 QK+V projections dribble into b0's early blocks
              # (qkproj slices first -- b1 attention needs them at idx 4 --
              # then V groups, one item per quad)
              fill_items = [lambda s=s: qk_proj(NMB + s) for s in range(NMB)]
              fill_items += [lambda jg=jg: v_proj_group(NJ // 4 + jg)
                             for jg in range(NJ // 4)]
              f_ctr = [0]

              def v1_fill():   # one item per quad: 40 items over 5 blocks
                  if f_ctr[0] < len(fill_items):
                      fill_items[f_ctr[0]]()
                      f_ctr[0] += 1

              b0_items = [lambda s=s: qk_proj(s) for s in range(4, NMB)]
              b0_ctr = [0]

              def b0_fill():   # rest of batch-0 QK proj inside block 0
                  if b0_ctr[0] < len(b0_items):
                      b0_items[b0_ctr[0]]()
                      b0_ctr[0] += 1

              # block order: one phase of 8 blocks per A2A piece; piece p
              # covers cols [off % 512, +w) of the even (off < 512) or odd
              # m-blocks (pieces must not straddle the even/odd boundary)
              order = []
              off = 0
              for p, wp in enumerate(PIECEW):
                  par, c0 = off // MB, off % MB
                  order += [(b, mb, c0, wp, p)
                            for b in range(B) for mb in range(par, NMB, 2)]
                  off += wp
              if nblk is not None:
                  order = order[:nblk]

              entries = []        # (qk, av)
              posts = {}          # stream idx -> [fns], run at av-flush
              deferred = []       # (target_idx, fn) resolved after build
              nblocks = len(order)
              nphase = len(PIECEW)
              # mid-stream out-projection spreading: piece p's collective
              # has landed by block 8(p+1)+6; one chunk per block keeps the
              # PSUM chain off the quad pipeline's critical path. The last
              # two pieces' projections run in the drain.
              op_items = {p: outproj_items(p) for p in range(nphase - 2)}
              op_spread = {p: min(8 * (p + 1) + 6, nblocks - 5)
                           for p in op_items}
              for idx, (b, mb, c0, w, piece) in enumerate(order):
                  fillfn = None
                  if idx == 0:
                      fillfn = b0_fill
                  elif idx in (1, 2, 3, 4, 5):
                      fillfn = v1_fill
                  qks, avs, tail_a, tail_b = mk_block(b, mb, c0, w, piece,
                                                      fill=fillfn)
                  entries.extend(zip(qks, avs))
                  P = len(entries) - 1
                  posts.setdefault(P, []).append(tail_a)
                  late = [tail_b]
                  if stages == 'full':
                      if idx % 8 == 7 and idx != nblocks - 1:
                          late.append(lambda p=idx // 8: a2a_piece(p))
                      elif idx == nblocks - 1:
                          late.append(lambda: (
                              a2a_piece(nphase - 1),
                              [f() for f in outproj_items(nphase - 2,
                                                          drain=True)],
                              pe_warm(4),
                              [f() for f in outproj_items(nphase - 1,
                                                          drain=True)]))
                  deferred.append((P + 2, late))
                  if stages == 'full':
                      for p, s0 in op_spread.items():
                          if s0 <= idx < s0 + 4:
                              deferred.append((P + 2,
                                               [op_items[p][idx - s0]]))

              final_posts = []
              last = len(entries) - 1
              for tgt, fns in deferred:
                  if tgt <= last:
                      posts.setdefault(tgt, []).extend(fns)
                  else:
                      final_posts.extend(fns)

              from collections import deque
              pending = deque()

              def flush_one():
                  pav, pe4, pafter = pending.popleft()
                  pav(pe4)
                  for fn in pafter:
                      fn()

              for i, (qk, av) in enumerate(entries):
                  e4 = qk()
                  if len(pending) >= lag:
                      flush_one()
                  pending.append((av, e4, posts.get(i, [])))
              while pending:
                  flush_one()
              for fn in final_posts:
                  fn()

    nc.compile()
    return nc


def _prep_inputs(x, Wq, Wk, Wv, Wo, bo):
    x = np.asarray(x, np.float32)
    Wq = np.asarray(Wq, np.float32)
    Wk = np.asarray(Wk, np.float32)
    Wv = np.asarray(Wv, np.float32)
    Wo = np.asarray(Wo, np.float32)
    bo = np.asarray(bo, np.float32)

    xT = np.ascontiguousarray(x.reshape(R, C).T).astype(BF16)
    woT = np.ascontiguousarray(Wo.T).astype(BF16)
    bias = np.ascontiguousarray(bo.reshape(4, 128).T).astype(np.float32)
    ident = np.eye(128, dtype=BF16)

    in_maps = []
    for h in range(NCORES):
        sl = slice(h * D, (h + 1) * D)
        wqk = np.concatenate(
            [Wq[sl].T, Wk[sl].T], axis=1).astype(BF16)
        wv = np.ascontiguousarray(Wv[sl].T).astype(BF16)
        in_maps.append({
            "xT": xT,
            "wqk": np.ascontiguousarray(wqk),
            "wv": wv,
            "wo": woT,
            "bias": bias,
            "ident": ident,
        })
    return in_maps


def kernel(x, Wq, Wk, Wv, Wo, bo, _want_results=False, _trace=False):
    from concourse import bass_utils

    if "nc" not in _CACHE:
        _CACHE["nc"] = _build(1)
    nc = _CACHE["nc"]

    in_maps = _prep_inputs(x, Wq, Wk, Wv, Wo, bo)
    res = bass_utils.run_bass_kernel_spmd(
        nc, in_maps, core_ids=list(range(NCORES)), trace=_trace)

    outT = np.concatenate(
        [np.asarray(res.results[j]["out"]) for j in range(NCORES)], axis=1)
    full = np.ascontiguousarray(outT.T).reshape(B, N, C).astype(np.float32)
    if _want_results:
        return full, res
    return full


def bench(x, Wq, Wk, Wv, Wo, bo, iters=8, reps=3, body_reps=1, nc=None):
    """Measure per-NEFF-execution time by chaining `iters` executions in one
    jit (output of exec i feeds the donated out-buffer operand of exec i+1),
    so per-exec time = (t_chain(iters) - t_chain(1)) / (iters - 1)."""
    import time
    import jax
    from jax.experimental.shard_map import shard_map
    from jax.sharding import Mesh, PartitionSpec
    from concourse import bass2jax, mybir

    if nc is None:
        key = ("nc", body_reps)
        if key not in _CACHE:
            _CACHE[key] = _build(body_reps)
        nc = _CACHE[key]
    bass2jax.install_neuronx_cc_hook()

    in_maps = _prep_inputs(x, Wq, Wk, Wv, Wo, bo)

    pname = nc.partition_id_tensor.name if nc.partition_id_tensor else None
    in_names, out_names, out_avals = [], [], []
    for alloc in nc.m.functions[0].allocations:
        if not isinstance(alloc, mybir.MemoryLocationSet):
            continue
        name = alloc.memorylocations[0].name
        if alloc.kind == "ExternalInput":
            if name != pname:
                in_names.append(name)
        elif alloc.kind == "ExternalOutput":
            out_names.append(name)
            out_avals.append(jax.core.ShapedArray(
                tuple(alloc.tensor_shape), mybir.dt.np(alloc.dtype)))
    n_params = len(in_names)
    all_names = in_names + out_names + ([pname] if pname else [])

    def _body(*args):
        ins = list(args[:n_params])
        outs = list(args[n_params:])
        extra = [bass2jax.partition_id_tensor()] if pname else []
        outs = list(bass2jax._bass_exec_p.bind(
            *ins, *outs, *extra,
            out_avals=tuple(out_avals),
            in_names=tuple(all_names),
            out_names=tuple(out_names),
            lowering_input_output_aliases=(),
            sim_require_finite=True,
            sim_require_nnan=True,
            nc=nc))
        return tuple(outs)

    devices = jax.devices()[:NCORES]
    mesh = Mesh(np.asarray(devices), ("core",))
    specs = (PartitionSpec("core"),) * (n_params + len(out_names))
    ospecs = (PartitionSpec("core"),) * len(out_names)
    fn = jax.jit(shard_map(_body, mesh=mesh, in_specs=specs,
                           out_specs=ospecs, check_rep=False))

    concat_in = [np.concatenate([np.asarray(in_maps[c][n])[None]
                                 for c in range(NCORES)], axis=0)
                 .reshape(NCORES * in_maps[0][n].shape[0],
                          *in_maps[0][n].shape[1:])
                 for n in in_names]
    concat_zero = [np.zeros((NCORES * a.shape[0], *a.shape[1:]), a.dtype)
                   for a in out_avals]
    dev_in = [jax.device_put(a) for a in concat_in]
    dev_zero = [jax.device_put(a) for a in concat_zero]

    fn(*dev_in, *dev_zero)[0].block_until_ready()  # compile+warm

    def chain(k):
        outs = tuple(dev_zero)
        t0 = time.perf_counter()
        for _ in range(k):
            outs = fn(*dev_in, *outs)
        outs[0].block_until_ready()
        return time.perf_counter() - t0

    ts = [chain(iters) for _ in range(reps)]
    t = min(ts)
    print(f"body_reps={body_reps} chain k={iters}: min {t*1e6:.0f} us")
    return t


# revision 49
# speedup vs baseline: 1.3023x; 1.0028x over previous
"""Distributed multi-head attention forward on 8 TRN2 NeuronCores.

Problem (hardcoded): x [2, 4096, 512] f32, Wq/Wk/Wv/Wo [512, 512], bo [512].
reference: torch-style MHA with 8 heads of dim 64, softmax scale 1/8.

Sharding: head-parallel. Core h computes head h for BOTH batches:
  - host sends x^T [512, 8192] (bf16) + per-head weight slices (pre-transposed)
  - Q^T/K^T [64, 8192] computed on-chip, duplicated into both partition
    halves so QK^T (contract dim = head_dim 64) runs as two concurrent
    row-tiled matmuls (tile_position (0,0)/(64,0))
  - S^T [j, m] orientation; exp is split across ScalarE (native Exp, fused
    scale) and VectorE (Schraudolph bit trick: one mult+add rounded into
    int16 == bf16 bits of exp, ~1.8% rms on those chunks), greedily
    balanced against each engine's other duties
  - AV runs TRANSPOSED: exp output e4 [j,m] is the 128x128 stationary
    operand, [V | ones] [128, 65] is the moving operand, accumulating
    ctx^T [m, d+1] in PSUM (65-cycle matmuls instead of 512)
  - normalize: denominator is per-PARTITION in ctx^T, so a reciprocal +
    per-partition tensor_scalar multiply normalizes directly (no DRAM
    broadcast bounce); a PE transpose (vs host-fed identity) restores
    [d, m] for A2A staging
  - AllToAll over all 8 cores reshards head-split -> row-split in THREE
    pieces (even m-blocks [512 wide], then odd blocks in two 256-wide
    column phases), so each collective starts as soon as its phase's
    blocks finish and the post-attention tail is one small piece
    (collective cost = 15us constant + size/40GBps)
  - out-proj per piece: full Wo^T on its rows + bias; host concatenates.

Scheduling (guided by TimelineSim engine-occupancy traces + HW bisects):
  - x streams in 512-col chunks after the weight DMAs; batch-0 first
  - QK-projection for batch 0 runs up front; batch-1 QK/V projections are
    emitted one item per quad inside batch-0's early blocks, with V
    projected 4 chunks per PSUM tile so the fill chain (matmuls -> evict
    -> next alloc through the one free PSUM buf) never head-blocks the
    in-order PE queue
  - flat quad stream: AV lags QK by `lag` quads; block tails are split
    (normalize at +0, transpose/stage at +2 quads) so the PE never waits
    on the DVE normalize
  - narrow blocks take proportionally more j-chunks per exp quad (fd
    stays 1024) and alternate QK row-halves per chunk-GROUP: per-chunk
    ping-pong of tile_position at <512 moving cols hard-faults the HW
  - each block's ctx^T and its transposed staging view share one PSUM
    bank (bf16 bitcast of the upper half), and only the block's first AV
    matmul sets start=True -- start zeroes the whole 2KB zero-region
  - mid-stream out-projections are emitted one chunk per block; the
    out-proj of the last two pieces runs in the drain
  - small DMAs (Q/K duplication) ride the otherwise-idle GpSimd queue.
"""

import numpy as np
import ml_dtypes

B, N, C = 2, 4096, 512
H, D = 8, 64
R = B * N            # 8192 global rows
NCORES = 8
MROWS = R // NCORES  # 1024 rows owned per core after A2A
BF16 = ml_dtypes.bfloat16

_CACHE = {}

# A2A piece column phases (widths must be multiples of 128): piece p covers
# out columns [sum(PIECEW[:p]), +PIECEW[p]) of each core's 1024 rows; uniform
# 256-wide phases let every collective start early and keep exp instructions
# full-width
PIECEW = (512, 256, 256)


def _build(reps=1, stages='full', quad=2, s4bufs=3, ctxbufs=2, ebufs=9,
           lag=5, warmn=2, dve_bias=0.0, nblk=None, debug_blk=None):
    import concourse.bass as bass
    import concourse.tile as tile
    from concourse import bacc, mybir

    import math
    fp32 = mybir.dt.float32
    bf16 = mybir.dt.bfloat16
    i16 = mybir.dt.int16
    SCH_A = float(0.125 * 128.0 / math.log(2.0))   # fold softmax scale
    SCH_B = float(127 * 128 - 0.0579615 * 128)
    AF = mybir.ActivationFunctionType

    nc = bacc.Bacc("TRN2", target_bir_lowering=False, debug=False,
                   num_devices=NCORES)

    xT = nc.dram_tensor("xT", [C, R], bf16, kind="ExternalInput").ap()
    wqk = nc.dram_tensor("wqk", [C, 128], bf16, kind="ExternalInput").ap()
    wv = nc.dram_tensor("wv", [C, D], bf16, kind="ExternalInput").ap()
    wo = nc.dram_tensor("wo", [C, C], bf16, kind="ExternalInput").ap()
    bias = nc.dram_tensor("bias", [128, 4], fp32, kind="ExternalInput").ap()
    ident = nc.dram_tensor("ident", [128, 128], bf16,
                           kind="ExternalInput").ap()
    out = nc.dram_tensor("out", [C, MROWS], fp32, kind="ExternalOutput").ap()

    KC = C // 128          # 4 contraction chunks of 128 over C
    NJ = N // 128          # 32 key chunks per batch
    MB = 512               # base m-block width
    NMB = N // MB          # 8 m-blocks per batch
    QUAD = quad            # j-chunks per exp batch

    # running busy-time estimates for the exp-engine balancer (ns)
    eng_busy = {"A": 1283.0, "D": 0.0}   # ACT starts with the table load

    def act_cost(fd):
        return (fd + 222) * 0.8333

    def dve_cost(fd):
        return (fd + 120) * 1.0417

    with tile.TileContext(nc) as tc:
        with (
            tc.tile_pool(name="xpool", bufs=4) as xpool,
            tc.tile_pool(name="wpool", bufs=1) as wpool,
            tc.tile_pool(name="qk", bufs=1) as qkpool,
            tc.tile_pool(name="vpool", bufs=1) as vpool,
            tc.tile_pool(name="epool", bufs=ebufs) as epool,
            tc.tile_pool(name="stage", bufs=3) as stpool,
            tc.tile_pool(name="misc", bufs=3) as miscpool,
            tc.tile_pool(name="capool", bufs=8) as capool,
            tc.tile_pool(name="ps_s4", bufs=s4bufs, space="PSUM") as ps_s4,
            tc.tile_pool(name="ps_ctx", bufs=ctxbufs, space="PSUM") as ps_ctx,
            tc.tile_pool(name="dram", bufs=1, space="DRAM") as dram,
        ):
          for _rep in range(reps):
            # ---- load inputs ----
              xt = []
              for k in range(KC):
                  t = xpool.tile([128, R], bf16, tag="xt")
                  xt.append(t)
              wqk_sb = wpool.tile([128, KC, 128], bf16, tag="wqk")
              nc.sync.dma_start(
                  wqk_sb[:], wqk.rearrange("(k p) m -> p k m", p=128))
              wv_sb = wpool.tile([128, KC, D], bf16, tag="wv")
              nc.sync.dma_start(
                  wv_sb[:], wv.rearrange("(k p) m -> p k m", p=128))
              id_sb = wpool.tile([128, 128], bf16, tag="ident")
              nc.sync.dma_start(id_sb[:], ident)
              XCH = 1024            # x load granularity (columns)
              for c0 in range(0, R, XCH):   # batch-0 chunks land first
                  for k in range(KC):
                      nc.sync.dma_start(
                          xt[k][:, c0:c0 + XCH],
                          xT[k * 128:(k + 1) * 128, c0:c0 + XCH])
              # out-proj weights aren't needed until much later; keep their
              # (slow, strided) loads off the Pool queue that carries the
              # early Q/K duplication DMAs
              wo_sb = wpool.tile([128, KC, C], bf16, tag="wo")
              nc.sync.dma_start(
                  wo_sb[:], wo.rearrange("(k p) m -> p k m", p=128))
              bias_sb = wpool.tile([128, 4], fp32, tag="bias")
              nc.sync.dma_start(bias_sb[:], bias)

              # PE HAM warm-up: the clock gate holds the PE at 1.2 GHz until
              # ~3.4us of sustained activity. Burn dummy matmuls (on a
              # memset tile, result discarded) while waiting on DMAs /
              # collectives so real matmuls run at 2.4 GHz; the memset
              # source lets the PE start at t~0 instead of after the first
              # weight DMA lands.
              wrm_src = miscpool.tile([128, MB], bf16, tag="wsrc")
              nc.vector.memset(wrm_src[:], 0.5)

              def pe_warm(n):
                  wrm = ps_ctx.tile([128, MB], fp32, tag="ctx")
                  for _ in range(n):
                      nc.tensor.matmul(
                          wrm[:], wrm_src[:, 0:128], wrm_src[:],
                          start=True, stop=True, skip_group_check=True)

              if warmn:
                  pe_warm(warmn)   # sized to fit inside the x-DMA wait

              # ---- QK projection: psum = [Q^T (parts 0:64); K^T (parts 64:128)]
              qt2 = qkpool.tile([128, R], bf16, tag="qt2")   # Q^T in both halves
              kt2 = qkpool.tile([128, R], bf16, tag="kt2")   # K^T in both halves
              def qk_proj(ms):
                  # one 512-wide slice: project, evict both halves (ScalarE;
                  # VectorE is loaded with exp work), then duplicate this
                  # slice into the opposite partition halves
                  ps = ps_ctx.tile([128, MB], fp32, tag="ctx")
                  for k in range(KC):
                      nc.tensor.matmul(
                          ps[:], wqk_sb[:, k, :],
                          xt[k][:, ms * MB:(ms + 1) * MB],
                          start=(k == 0), stop=(k == KC - 1))
                  sl = slice(ms * MB, (ms + 1) * MB)
                  nc.scalar.copy(qt2[0:64, sl], ps[0:64, :])
                  nc.scalar.copy(kt2[64:128, sl], ps[64:128, :])
                  eng_busy["A"] += 2 * act_cost(MB)
                  nc.gpsimd.dma_start(qt2[64:128, sl], qt2[0:64, sl])
                  nc.gpsimd.dma_start(kt2[0:64, sl], kt2[64:128, sl])

              for ms in range(4):       # slices 4..7 + batch 1 are JIT'd
                  qk_proj(ms)

              # ---- V storage: V natural [j, 64] + ones column (col 64).
              vst = vpool.tile([128, 2 * NJ, D + 1], bf16, tag="vst")
              nc.vector.memset(vst[:, :, D:D + 1], 1.0)

              def v_proj_group(jg):
                  # project 4 j-chunks into one PSUM tile + one strided
                  # eviction: a single-tile chain (matmuls -> evict -> next
                  # alloc) is 4x shorter than per-chunk tiles and stops the
                  # fills from head-blocking the in-order PE queue
                  psv = ps_ctx.tile([128, MB], fp32, tag="ctx")
                  for jj in range(4):
                      jc = jg * 4 + jj
                      ps = psv[:, jj * D:(jj + 1) * D]
                      for k in range(KC):
                          nc.tensor.matmul(
                              ps[:], xt[k][:, jc * 128:(jc + 1) * 128],
                              wv_sb[:, k, :],
                              start=(k == 0), stop=(k == KC - 1))
                  nc.scalar.copy(vst[:, jg * 4:(jg + 1) * 4, 0:D],
                                 psv[:, 0:4 * D])
                  eng_busy["A"] += act_cost(4 * D)

              if stages == 'proj':
                  for jg in range(2 * NJ // 4):
                      v_proj_group(jg)
                  continue

              # ---- A2A pieces ----
              a2a = [dram.tile([R // 16, w], bf16, name=f"a2a_in{i}")
                     for i, w in enumerate(PIECEW)]
              a2a_o = [dram.tile([R // 16, w], bf16, name=f"a2a_out{i}")
                       for i, w in enumerate(PIECEW)]
              if debug_blk is not None:
                  dbg_e4 = dram.tile([128, 1024], bf16, name="dbg_e4")
                  dbg_ctx = dram.tile([128, 260], fp32, name="dbg_ctx")
                  dbg_st = dram.tile([128, 256], bf16, name="dbg_st")

              def mk_block(b, mb, c0, w, piece, fill=None):
                  # Returns (qk_thunks, av_thunks, tail_a, tail_b). Block
                  # covers query columns [mb*MB+c0, mb*MB+c0+w) of batch b.
                  # Narrow blocks take more j-chunks per exp so every exp
                  # instruction stays [128, quad*512] (per-inst overhead
                  # would otherwise dominate the 128-wide phases).
                  base = b * N + mb * MB + c0
                  nsub = w // 128
                  QN = (quad * MB) // w      # j-chunks per exp quad
                  state = {"ctx": None, "first": True}

                  def get_ctx():
                      if state["ctx"] is None:
                          # one PSUM bank per block: fp32 cols 0:260 hold
                          # ctx^T [128 m, 4x65]; the bank's upper half
                          # (bf16 view) receives the PE-transposed [d, m]
                          ctxf = ps_ctx.tile([128, MB], fp32, tag="ctx")
                          state["ctx"] = ctxf
                      return state["ctx"]

                  def mk_qk(q0, nq):
                      def qk():
                          s4 = ps_s4.tile([128, QUAD * MB], fp32, tag="s4")
                          grp = max(1, MB // w)
                          for qi in range(nq):
                              jc = q0 + qi
                              half = 64 * ((jc // grp) % 2)
                              # consecutive chunk-groups use opposite PE
                              # row-halves so adjacent groups can run
                              # concurrently (incl. across quad boundaries);
                              # grouping keeps the half-switch period at
                              # >=512 moving cols -- per-chunk ping-pong at
                              # 256-wide hard-faults the device
                              jsl = slice(b * N + jc * 128,
                                          b * N + (jc + 1) * 128)
                              nc.tensor.matmul(
                                  s4[:, qi * w:(qi + 1) * w],
                                  kt2[half:half + 64, jsl],
                                  qt2[half:half + 64, base:base + w],
                                  start=True, stop=True,
                                  tile_position=(half, 0))
                          # exp: greedily balance ScalarE vs VectorE busy
                          fd = nq * w
                          ca, cd = act_cost(fd), dve_cost(fd)
                          on_dve = (eng_busy["D"] + cd + dve_bias
                                    < eng_busy["A"] + ca)
                          if on_dve:
                              eng_busy["D"] += cd
                              e4i = epool.tile([128, QUAD * MB], i16,
                                               tag="e4")
                              nc.vector.tensor_scalar(
                                  out=e4i[:, 0:fd],
                                  in0=s4[:, 0:fd],
                                  scalar1=SCH_A, scalar2=SCH_B,
                                  op0=mybir.AluOpType.mult,
                                  op1=mybir.AluOpType.add)
                              e4 = e4i[:].bitcast(bf16)
                          else:
                              eng_busy["A"] += ca
                              e4t = epool.tile([128, QUAD * MB], bf16,
                                               tag="e4")
                              e4 = e4t[:]
                              nc.scalar.activation(e4[:, 0:fd],
                                                   s4[:, 0:fd], AF.Exp,
                                                   scale=0.125)
                          if b == 0 and mb == 0 and c0 == 0 and q0 % 4 == 0:
                              v_proj_group(q0 // 4)  # JIT V proj, batch 0
                          if fill is not None:
                              fill()
                          if debug_blk == (b, mb, c0) and q0 == 0:
                              nc.sync.dma_start(dbg_e4[:, 0:fd],
                                                e4[:, 0:fd])
                          return e4
                      return qk

                  def mk_av(q0, nq):
                      def av(e4):
                          if stages == 'qkexp':
                              return
                          ctx = get_ctx()
                          for qi in range(nq):
                              jc = q0 + qi
                              vsl = vst[:, b * NJ + jc, :]
                              for sub in range(nsub):
                                  # start=True zeroes the ENTIRE 2KB psum
                                  # zero-region, so only the block's very
                                  # first AV matmul may set it; the other
                                  # subs' first writes land on the pending-
                                  # zero region and overwrite cleanly
                                  nc.tensor.matmul(
                                      ctx[:, sub * 65:sub * 65 + 65],
                                      e4[:, qi * w + sub * 128:
                                         qi * w + (sub + 1) * 128],
                                      vsl,
                                      start=state["first"],
                                      stop=(jc == NJ - 1),
                                      skip_group_check=True)
                                  state["first"] = False
                      return av

                  qks, avs = [], []
                  for q0 in range(0, NJ, QN):
                      nq = min(QN, NJ - q0)
                      qks.append(mk_qk(q0, nq))
                      avs.append(mk_av(q0, nq))

                  def tail_a():
                      if stages in ('qkexp', 'noav', 'notail'):
                          return
                      if debug_blk == (b, mb, c0):
                          dbc = stpool.tile([128, 260], fp32, tag="dbg")
                          nc.vector.tensor_copy(
                              dbc[:], state["ctx"][:, 0:260])
                          nc.sync.dma_start(dbg_ctx[:], dbc[:])
                      ctx = state["ctx"]
                      # normalize in ctx^T orientation: denominator is
                      # per-partition -> reciprocal + per-partition multiply
                      st = stpool.tile([128, nsub * D], bf16, tag="st")
                      state["st"] = st
                      for sub in range(nsub):
                          rc = miscpool.tile([128, 1], fp32, tag="rc")
                          nc.vector.reciprocal(
                              rc[:], ctx[:, sub * 65 + D:sub * 65 + D + 1])
                          nc.vector.tensor_scalar_mul(
                              st[:, sub * D:(sub + 1) * D],
                              ctx[:, sub * 65:sub * 65 + D], rc[:, 0:1])
                          eng_busy["D"] += 126 + dve_cost(D)

                  def tail_b():
                      if stages in ('qkexp', 'noav', 'notail', 'taila'):
                          return
                      # runs two quads later so the PE queue never waits on
                      # the DVE normalize: transpose back to [d, m] via PE
                      # (vs identity) into the ctx bank's upper half, evict,
                      # stage to the A2A buffer
                      ctx, st = state["ctx"], state["st"]
                      pt = ctx[0:64, 256:512].bitcast(bf16)  # [64, 512]
                      for sub in range(nsub):
                          nc.tensor.transpose(
                              pt[:, sub * 128:(sub + 1) * 128],
                              st[:, sub * D:(sub + 1) * D], id_sb[:])
                      sg = stpool.tile([64, w], bf16, tag="sg")
                      nc.scalar.copy(sg[:], pt[:, 0:w])
                      eng_busy["A"] += act_cost(w)
                      if debug_blk == (b, mb, c0):
                          nc.sync.dma_start(dbg_st[:, 0:nsub * D], st[:])
                      # shard s owns global rows [s*1024, (s+1)*1024)
                      s = (b * NMB + mb) // 2
                      nc.sync.dma_start(
                          a2a[piece][s * 64:(s + 1) * 64, :], sg[:])
                  return qks, avs, tail_a, tail_b

              def a2a_piece(p):
                  nc.gpsimd.collective_compute(
                      "AllToAll", mybir.AluOpType.bypass,
                      replica_groups=[list(range(NCORES))],
                      ins=[a2a[p].opt()], outs=[a2a_o[p].opt()])

              # out-column ranges per piece (even block 0:512, odd 512:1024)
              PIECE_OUT = []
              off = 0
              for i, w in enumerate(PIECEW):
                  PIECE_OUT.append((off, w))
                  off += w

              def outproj_items(p, drain=False):
                  # split into 5 thunks (ca loads + one per out-chan chunk)
                  # so the single-free-PSUM-buf chain never head-blocks the
                  # in-order PE queue for more than ~1 quad. In the drain
                  # (last pieces) the ACT/DVE queues are idle: spread the
                  # DMA issue across them so 625ns HWDGE issues overlap.
                  o0, w = PIECE_OUT[p]
                  ca = []
                  qs = [nc.sync] * 4

                  def loads():
                      for k in range(KC):
                          t = capool.tile([128, MB], bf16, tag="ca")
                          qs[k].dma_start(
                              t[:, 0:w], a2a_o[p][k * 128:(k + 1) * 128, :])
                          ca.append(t)

                  def mk_cc(cc):
                      def cc_fn():
                          ps = ps_ctx.tile([128, MB], fp32, tag="ctx")
                          for k in range(KC):
                              nc.tensor.matmul(
                                  ps[:, 0:w],
                                  wo_sb[:, k, cc * 128:(cc + 1) * 128],
                                  ca[k][:, 0:w], start=(k == 0),
                                  stop=(k == KC - 1))
                          ot = stpool.tile([128, MB], fp32, tag="ot")
                          nc.vector.tensor_scalar_add(
                              ot[:, 0:w], ps[:, 0:w], bias_sb[:, cc:cc + 1])
                          eng_busy["D"] += dve_cost(w)
                          qs[cc].dma_start(
                              out[cc * 128:(cc + 1) * 128, o0:o0 + w],
                              ot[:, 0:w])
                      return cc_fn

                  def first():
                      loads()
                      mk_cc(0)()
                  return [first] + [mk_cc(cc) for cc in range(1, KC)]

              def outproj_piece(p):
                  for fn in outproj_items(p):
                      fn()

              # batch-1 QK+V projections dribble into b0's early blocks
              # (qkproj slices first -- b1 attention needs them at idx 4 --
              # then V groups, one item per quad)
              fill_items = [lambda s=s: qk_proj(NMB + s) for s in range(NMB)]
              fill_items += [lambda jg=jg: v_proj_group(NJ // 4 + jg)
                             for jg in range(NJ // 4)]
              f_ctr = [0]

              def v1_fill():   # one item per quad: 40 items over 5 blocks
                  if f_ctr[0] < len(fill_items):
                      fill_items[f_ctr[0]]()
                      f_ctr[0] += 1

              b0_items = [lambda s=s: qk_proj(s) for s in range(4, NMB)]
              b0_ctr = [0]

              def b0_fill():   # rest of batch-0 QK proj inside block 0
                  if b0_ctr[0] < len(b0_items):
                      b0_items[b0_ctr[0]]()
                      b0_ctr[0] += 1

              # block order: one phase of 8 blocks per A2A piece; piece p
              # covers cols [off % 512, +w) of the even (off < 512) or odd
              # m-blocks (pieces must not straddle the even/odd boundary)
              order = []
              off = 0
              for p, wp in enumerate(PIECEW):
                  par, c0 = off // MB, off % MB
                  order += [(b, mb, c0, wp, p)
                            for b in range(B) for mb in range(par, NMB, 2)]
                  off += wp
              if nblk is not None:
                  order = order[:nblk]

              entries = []        # (qk, av)
              posts = {}          # stream idx -> [fns], run at av-flush
              deferred = []       # (target_idx, fn) resolved after build
              nblocks = len(order)
              nphase = len(PIECEW)
              # mid-stream out-projection spreading: piece p's collective
              # has landed by block 8(p+1)+6; one chunk per block keeps the
              # PSUM chain off the quad pipeline's critical path. The last
              # two pieces' projections run in the drain.
              op_items = {p: outproj_items(p) for p in range(nphase - 2)}
              op_spread = {p: min(8 * (p + 1) + 6, nblocks - 5)
                           for p in op_items}
              for idx, (b, mb, c0, w, piece) in enumerate(order):
                  fillfn = None
                  if idx == 0:
                      fillfn = b0_fill
                  elif idx in (1, 2, 3, 4, 5):
                      fillfn = v1_fill
                  qks, avs, tail_a, tail_b = mk_block(b, mb, c0, w, piece,
                                                      fill=fillfn)
                  entries.extend(zip(qks, avs))
                  P = len(entries) - 1
                  posts.setdefault(P, []).append(tail_a)
                  late = [tail_b]
                  if stages == 'full':
                      if idx % 8 == 7 and idx != nblocks - 1:
                          late.append(lambda p=idx // 8: a2a_piece(p))
                      elif idx == nblocks - 1:
                          late.append(lambda: (
                              a2a_piece(nphase - 1),
                              [f() for f in outproj_items(nphase - 2,
                                                          drain=True)],
                              pe_warm(4),
                              [f() for f in outproj_items(nphase - 1,
                                                          drain=True)]))
                  deferred.append((P + 2, late))
                  if stages == 'full':
                      for p, s0 in op_spread.items():
                          if s0 <= idx < s0 + 4:
                              deferred.append((P + 2,
                                               [op_items[p][idx - s0]]))

              final_posts = []
              last = len(entries) - 1
              for tgt, fns in deferred:
                  if tgt <= last:
                      posts.setdefault(tgt, []).extend(fns)
                  else:
                      final_posts.extend(fns)

              from collections import deque
              pending = deque()

              def flush_one():
                  pav, pe4, pafter = pending.popleft()
                  pav(pe4)
                  for fn in pafter:
                      fn()

              for i, (qk, av) in enumerate(entries):
                  e4 = qk()
                  if len(pending) >= lag:
                      flush_one()
                  pending.append((av, e4, posts.get(i, [])))
              while pending:
                  flush_one()
              for fn in final_posts:
                  fn()

    nc.compile()
    return nc


def _prep_inputs(x, Wq, Wk, Wv, Wo, bo):
    x = np.asarray(x, np.float32)
    Wq = np.asarray(Wq, np.float32)
    Wk = np.asarray(Wk, np.float32)
    Wv = np.asarray(Wv, np.float32)
    Wo = np.asarray(Wo, np.float32)
    bo = np.asarray(bo, np.float32)

    xT = np.ascontiguousarray(x.reshape(R, C).T).astype(BF16)
    woT = np.ascontiguousarray(Wo.T).astype(BF16)
    bias = np.ascontiguousarray(bo.reshape(4, 128).T).astype(np.float32)
    ident = np.eye(128, dtype=BF16)

    in_maps = []
    for h in range(NCORES):
        sl = slice(h * D, (h + 1) * D)
        wqk = np.concatenate(
            [Wq[sl].T, Wk[sl].T], axis=1).astype(BF16)
        wv = np.ascontiguousarray(Wv[sl].T).astype(BF16)
        in_maps.append({
            "xT": xT,
            "wqk": np.ascontiguousarray(wqk),
            "wv": wv,
            "wo": woT,
            "bias": bias,
            "ident": ident,
        })
    return in_maps


def kernel(x, Wq, Wk, Wv, Wo, bo, _want_results=False, _trace=False):
    from concourse import bass_utils

    if "nc" not in _CACHE:
        _CACHE["nc"] = _build(1)
    nc = _CACHE["nc"]

    in_maps = _prep_inputs(x, Wq, Wk, Wv, Wo, bo)
    res = bass_utils.run_bass_kernel_spmd(
        nc, in_maps, core_ids=list(range(NCORES)), trace=_trace)

    outT = np.concatenate(
        [np.asarray(res.results[j]["out"]) for j in range(NCORES)], axis=1)
    full = np.ascontiguousarray(outT.T).reshape(B, N, C).astype(np.float32)
    if _want_results:
        return full, res
    return full


def bench(x, Wq, Wk, Wv, Wo, bo, iters=8, reps=3, body_reps=1, nc=None):
    """Measure per-NEFF-execution time by chaining `iters` executions in one
    jit (output of exec i feeds the donated out-buffer operand of exec i+1),
    so per-exec time = (t_chain(iters) - t_chain(1)) / (iters - 1)."""
    import time
    import jax
    from jax.experimental.shard_map import shard_map
    from jax.sharding import Mesh, PartitionSpec
    from concourse import bass2jax, mybir

    if nc is None:
        key = ("nc", body_reps)
        if key not in _CACHE:
            _CACHE[key] = _build(body_reps)
        nc = _CACHE[key]
    bass2jax.install_neuronx_cc_hook()

    in_maps = _prep_inputs(x, Wq, Wk, Wv, Wo, bo)

    pname = nc.partition_id_tensor.name if nc.partition_id_tensor else None
    in_names, out_names, out_avals = [], [], []
    for alloc in nc.m.functions[0].allocations:
        if not isinstance(alloc, mybir.MemoryLocationSet):
            continue
        name = alloc.memorylocations[0].name
        if alloc.kind == "ExternalInput":
            if name != pname:
                in_names.append(name)
        elif alloc.kind == "ExternalOutput":
            out_names.append(name)
            out_avals.append(jax.core.ShapedArray(
                tuple(alloc.tensor_shape), mybir.dt.np(alloc.dtype)))
    n_params = len(in_names)
    all_names = in_names + out_names + ([pname] if pname else [])

    def _body(*args):
        ins = list(args[:n_params])
        outs = list(args[n_params:])
        extra = [bass2jax.partition_id_tensor()] if pname else []
        outs = list(bass2jax._bass_exec_p.bind(
            *ins, *outs, *extra,
            out_avals=tuple(out_avals),
            in_names=tuple(all_names),
            out_names=tuple(out_names),
            lowering_input_output_aliases=(),
            sim_require_finite=True,
            sim_require_nnan=True,
            nc=nc))
        return tuple(outs)

    devices = jax.devices()[:NCORES]
    mesh = Mesh(np.asarray(devices), ("core",))
    specs = (PartitionSpec("core"),) * (n_params + len(out_names))
    ospecs = (PartitionSpec("core"),) * len(out_names)
    fn = jax.jit(shard_map(_body, mesh=mesh, in_specs=specs,
                           out_specs=ospecs, check_rep=False))

    concat_in = [np.concatenate([np.asarray(in_maps[c][n])[None]
                                 for c in range(NCORES)], axis=0)
                 .reshape(NCORES * in_maps[0][n].shape[0],
                          *in_maps[0][n].shape[1:])
                 for n in in_names]
    concat_zero = [np.zeros((NCORES * a.shape[0], *a.shape[1:]), a.dtype)
                   for a in out_avals]
    dev_in = [jax.device_put(a) for a in concat_in]
    dev_zero = [jax.device_put(a) for a in concat_zero]

    fn(*dev_in, *dev_zero)[0].block_until_ready()  # compile+warm

    def chain(k):
        outs = tuple(dev_zero)
        t0 = time.perf_counter()
        for _ in range(k):
            outs = fn(*dev_in, *outs)
        outs[0].block_until_ready()
        return time.perf_counter() - t0

    ts = [chain(iters) for _ in range(reps)]
    t = min(ts)
    print(f"body_reps={body_reps} chain k={iters}: min {t*1e6:.0f} us")
    return t
